# revision 8
# baseline (speedup 1.0000x reference)
"""LoRA adapter kernel for 8 Trainium2 NeuronCores.

Computes out = x @ W^T + b + 2.0 * (x @ A^T) @ B^T  for
x [4,2048,4096], W [4096,4096], b [4096], A [16,4096], B [4096,16].

Strategy: data-parallel over tokens (8192 tokens / 8 cores = 1024 each).
Each core computes outT[o, m] = W_slab^T-style matmuls with the
contraction dim (d) on SBUF partitions:
  - x^T shard [4096, 1024] stays resident in SBUF (128 KB/partition)
  - W streamed per 128-wide output slab as contiguous [4096, 128] blocks
  - LoRA: xa^T[r, m] = (2A) @ x^T once, then accumulated into the main
    PSUM group via a K=17 matmul whose extra row is ones x bias (folds
    the bias add into the matmul for free).
Matmuls run in float32r (single-pass reduced-mantissa fp32, 4x the
throughput of exact fp32 on the PE; measured ~1.6e-4 rel err per
128-deep contraction) accumulating into fp32 PSUM.
"""

import numpy as np

B, S, D_IN, D_OUT, RANK = 4, 2048, 4096, 4096, 16
SCALING = 2.0  # alpha/rank = 32/16; exact power of two, folded into A
N_CORES = 8
M = (B * S) // N_CORES  # tokens per core = 1024
KT = D_IN // 128        # 32 contraction tiles
OT = D_OUT // 128       # 32 output-partition tiles
MH = M // 512           # 2 moving-dim halves (PSUM bank limit = 512 fp32)

_compiled = None


def _build():
    global _compiled
    if _compiled is not None:
        return _compiled

    import concourse.mybir as mybir
    import concourse.tile as tile
    from concourse import bacc

    f32 = mybir.dt.float32
    f32r = mybir.dt.float32r

    nc = bacc.Bacc(
        "TRN2", target_bir_lowering=False, debug=False, num_devices=N_CORES
    )

    xT_d = nc.dram_tensor("xT", [D_IN, M], f32r, kind="ExternalInput").ap()
    WT_d = nc.dram_tensor("WT", [OT, D_IN, 128], f32r, kind="ExternalInput").ap()
    AT_d = nc.dram_tensor("AT", [D_IN, RANK], f32r, kind="ExternalInput").ap()
    BT_d = nc.dram_tensor("BT", [RANK + 1, D_OUT], f32r, kind="ExternalInput").ap()
    ones_d = nc.dram_tensor("ones", [1, M], f32r, kind="ExternalInput").ap()
    outT_d = nc.dram_tensor("outT", [D_OUT, M], f32, kind="ExternalOutput").ap()

    with tile.TileContext(nc) as tc:
        with (
            tc.tile_pool(name="xt", bufs=1) as xt_pool,
            tc.tile_pool(name="wt", bufs=2 * KT) as wt_pool,
            tc.tile_pool(name="misc", bufs=1) as misc_pool,
            tc.tile_pool(name="ost", bufs=4) as out_pool,
            tc.tile_pool(name="psum", bufs=4, space="PSUM") as psum_pool,
            tc.tile_pool(name="psxa", bufs=2, space="PSUM") as psxa_pool,
        ):
            # Resident x^T shard: 32 tiles of [128, 1024].
            xt = []
            for kd in range(KT):
                t = xt_pool.tile([128, M], f32r, tag=f"xt{kd}")
                nc.sync.dma_start(out=t, in_=xT_d[kd * 128 : (kd + 1) * 128, :])
                xt.append(t)

            at = []
            for kd in range(KT):
                t = misc_pool.tile([128, RANK], f32r, tag=f"at{kd}")
                nc.sync.dma_start(out=t, in_=AT_d[kd * 128 : (kd + 1) * 128, :])
                at.append(t)

            bt = misc_pool.tile([RANK + 1, D_OUT], f32r, tag="bt")
            nc.sync.dma_start(out=bt, in_=BT_d)

            # Phase 1: xa^T [17, 1024]; row 16 = ones (bias row).
            # Row 16 = ones (the bias row), DMAed in: compute engines can't
            # address a partition range based at 16, and memset doesn't
            # lower for float32r, but DMA writes any partition range.
            xa = misc_pool.tile([RANK + 1, M], f32r, tag="xa")
            nc.sync.dma_start(out=xa[RANK : RANK + 1, :], in_=ones_d)
            for mh in range(MH):
                ps = psxa_pool.tile([RANK, 512], f32, tag="psxa")
                for kd in range(KT):
                    nc.tensor.matmul(
                        ps,
                        lhsT=at[kd],
                        rhs=xt[kd][:, mh * 512 : (mh + 1) * 512],
                        start=(kd == 0),
                        stop=(kd == KT - 1),
                    )
                nc.vector.tensor_copy(xa[0:RANK, mh * 512 : (mh + 1) * 512], ps)

            # Phase 2: main matmul, one 128-wide output slab at a time.
            for ot in range(OT):
                wtiles = []
                for kd in range(KT):
                    w = wt_pool.tile([128, 128], f32r, tag="w")
                    nc.sync.dma_start(
                        out=w, in_=WT_d[ot, kd * 128 : (kd + 1) * 128, :]
                    )
                    wtiles.append(w)
                pss = [
                    psum_pool.tile([128, 512], f32, tag="ps", name=f"ps{mh}")
                    for mh in range(MH)
                ]
                # Each weight tile loads once and feeds both m-halves.
                for kd in range(KT):
                    for mh in range(MH):
                        nc.tensor.matmul(
                            pss[mh],
                            lhsT=wtiles[kd],
                            rhs=xt[kd][:, mh * 512 : (mh + 1) * 512],
                            start=(kd == 0),
                            stop=False,
                        )
                for mh in range(MH):
                    nc.tensor.matmul(
                        pss[mh],
                        lhsT=bt[:, ot * 128 : (ot + 1) * 128],
                        rhs=xa[:, mh * 512 : (mh + 1) * 512],
                        start=False,
                        stop=True,
                    )
                    o_sb = out_pool.tile([128, 512], f32, tag="osb")
                    nc.vector.tensor_copy(o_sb, pss[mh])
                    nc.sync.dma_start(
                        out=outT_d[
                            ot * 128 : (ot + 1) * 128, mh * 512 : (mh + 1) * 512
                        ],
                        in_=o_sb,
                    )

    nc.compile()
    _compiled = nc
    return nc


def _prep_in_maps(x, W, b, lora_A, lora_B):
    xf = np.ascontiguousarray(np.asarray(x, np.float32)).reshape(B * S, D_IN)
    WT3 = np.ascontiguousarray(
        np.asarray(W, np.float32).reshape(OT, 128, D_IN).transpose(0, 2, 1)
    )
    AT2 = np.ascontiguousarray(np.asarray(lora_A, np.float32).T * SCALING)
    BT17 = np.ascontiguousarray(
        np.concatenate(
            [np.asarray(lora_B, np.float32).T, np.asarray(b, np.float32)[None, :]], 0
        )
    )

    in_maps = []
    for c in range(N_CORES):
        xT = np.ascontiguousarray(xf[c * M : (c + 1) * M].T)
        in_maps.append(
            {
                "xT": xT,
                "WT": WT3,
                "AT": AT2,
                "BT": BT17,
                "ones": np.ones((1, M), np.float32),
            }
        )
    return in_maps


def kernel(x, W, b, lora_A, lora_B):
    nc = _build()
    from concourse.bass_utils import run_bass_kernel_spmd

    in_maps = _prep_in_maps(x, W, b, lora_A, lora_B)
    res = run_bass_kernel_spmd(
        nc, in_maps, core_ids=list(range(N_CORES)), trace=False
    )

    out = np.empty((B * S, D_OUT), np.float32)
    for c in range(N_CORES):
        out[c * M : (c + 1) * M] = res.results[c]["outT"].T
    return out.reshape(B, S, D_OUT)


# revision 11
# speedup vs baseline: 1.4822x; 1.4822x over previous
"""LoRA adapter kernel for 8 Trainium2 NeuronCores.

Computes out = x @ W^T + b + 2.0 * (x @ A^T) @ B^T  for
x [4,2048,4096], W [4096,4096], b [4096], A [16,4096], B [4096,16].

Strategy: data-parallel over tokens (8192 tokens / 8 cores = 1024 each).
Each core computes outT[o, m] = W_slab^T-style matmuls with the
contraction dim (d) on SBUF partitions:
  - x^T shard [4096, 1024] stays resident in SBUF (128 KB/partition)
  - W streamed per 128-wide output slab as contiguous [4096, 128] blocks
  - LoRA: xa^T[r, m] = (2A) @ x^T once, then accumulated into the main
    PSUM group via a K=17 matmul whose extra row is ones x bias (folds
    the bias add into the matmul for free).
Matmuls run in float32r (single-pass reduced-mantissa fp32, 4x the
throughput of exact fp32 on the PE; measured ~1.6e-4 rel err per
128-deep contraction) accumulating into fp32 PSUM.
"""

import numpy as np

B, S, D_IN, D_OUT, RANK = 4, 2048, 4096, 4096, 16
SCALING = 2.0  # alpha/rank = 32/16; exact power of two, folded into A
N_CORES = 8
M = (B * S) // N_CORES  # tokens per core = 1024
KT = D_IN // 128        # 32 contraction tiles
OT = D_OUT // 128       # 32 output-partition tiles
MH = M // 512           # 2 moving-dim halves (PSUM bank limit = 512 fp32)

_compiled = None


def _build():
    global _compiled
    if _compiled is not None:
        return _compiled

    import concourse.mybir as mybir
    import concourse.tile as tile
    from concourse import bacc

    f32 = mybir.dt.float32
    f32r = mybir.dt.float32r

    nc = bacc.Bacc(
        "TRN2", target_bir_lowering=False, debug=False, num_devices=N_CORES
    )

    xT_d = nc.dram_tensor("xT", [D_IN, M], f32r, kind="ExternalInput").ap()
    WT_d = nc.dram_tensor("WT", [OT, D_IN, 128], f32r, kind="ExternalInput").ap()
    AT_d = nc.dram_tensor("AT", [D_IN, RANK], f32r, kind="ExternalInput").ap()
    BT_d = nc.dram_tensor("BT", [RANK + 1, D_OUT], f32r, kind="ExternalInput").ap()
    ones_d = nc.dram_tensor("ones", [1, M], f32r, kind="ExternalInput").ap()
    outT_d = nc.dram_tensor("outT", [D_OUT, M], f32, kind="ExternalOutput").ap()

    with tile.TileContext(nc) as tc:
        with (
            tc.tile_pool(name="xt", bufs=1) as xt_pool,
            tc.tile_pool(name="wt", bufs=3) as wt_pool,
            tc.tile_pool(name="misc", bufs=1) as misc_pool,
            tc.tile_pool(name="ost", bufs=4) as out_pool,
            tc.tile_pool(name="psum", bufs=4, space="PSUM") as psum_pool,
            tc.tile_pool(name="psxa", bufs=2, space="PSUM") as psxa_pool,
        ):
            # Resident x^T shard: 32 tiles of [128, 1024].
            xt = []
            for kd in range(KT):
                t = xt_pool.tile([128, M], f32r, tag=f"xt{kd}")
                nc.sync.dma_start(out=t, in_=xT_d[kd * 128 : (kd + 1) * 128, :])
                xt.append(t)

            at = []
            for kd in range(KT):
                t = misc_pool.tile([128, RANK], f32r, tag=f"at{kd}")
                nc.sync.dma_start(out=t, in_=AT_d[kd * 128 : (kd + 1) * 128, :])
                at.append(t)

            bt = misc_pool.tile([RANK + 1, D_OUT], f32r, tag="bt")
            nc.sync.dma_start(out=bt, in_=BT_d)

            # Phase 1: xa^T [17, 1024]; row 16 = ones (bias row).
            # Row 16 = ones (the bias row), DMAed in: compute engines can't
            # address a partition range based at 16, and memset doesn't
            # lower for float32r, but DMA writes any partition range.
            xa = misc_pool.tile([RANK + 1, M], f32r, tag="xa")
            nc.sync.dma_start(out=xa[RANK : RANK + 1, :], in_=ones_d)
            for mh in range(MH):
                ps = psxa_pool.tile([RANK, 512], f32, tag="psxa")
                for kd in range(KT):
                    nc.tensor.matmul(
                        ps,
                        lhsT=at[kd],
                        rhs=xt[kd][:, mh * 512 : (mh + 1) * 512],
                        start=(kd == 0),
                        stop=(kd == KT - 1),
                    )
                nc.vector.tensor_copy(xa[0:RANK, mh * 512 : (mh + 1) * 512], ps)

            # Phase 2: main matmul, one 128-wide output slab at a time.
            # The whole 2 MB W slab moves in ONE strided DMA (slab is
            # [4096, 128] contiguous in DRAM; SBUF holds it as
            # [128 (d_lo), 32*128 (kd, o)] with 512 B runs) — issuing 32
            # separate 64 KB DMAs saturates the sync engine's issue rate
            # and starves the PE.
            for ot in range(OT):
                w = wt_pool.tile([128, KT * 128], f32r, tag="w", name="w")
                nc.sync.dma_start(
                    out=w.rearrange("p (kd o) -> p kd o", kd=KT),
                    in_=WT_d[ot].rearrange("(kd p) o -> p kd o", p=128),
                )
                pss = [
                    psum_pool.tile([128, 512], f32, tag="ps", name=f"ps{mh}")
                    for mh in range(MH)
                ]
                # Each weight tile loads once and feeds both m-halves.
                for kd in range(KT):
                    for mh in range(MH):
                        nc.tensor.matmul(
                            pss[mh],
                            lhsT=w[:, kd * 128 : (kd + 1) * 128],
                            rhs=xt[kd][:, mh * 512 : (mh + 1) * 512],
                            start=(kd == 0),
                            stop=False,
                        )
                for mh in range(MH):
                    nc.tensor.matmul(
                        pss[mh],
                        lhsT=bt[:, ot * 128 : (ot + 1) * 128],
                        rhs=xa[:, mh * 512 : (mh + 1) * 512],
                        start=False,
                        stop=True,
                    )
                    o_sb = out_pool.tile([128, 512], f32, tag="osb")
                    nc.vector.tensor_copy(o_sb, pss[mh])
                    nc.sync.dma_start(
                        out=outT_d[
                            ot * 128 : (ot + 1) * 128, mh * 512 : (mh + 1) * 512
                        ],
                        in_=o_sb,
                    )

    nc.compile()
    _compiled = nc
    return nc


def _prep_in_maps(x, W, b, lora_A, lora_B):
    xf = np.ascontiguousarray(np.asarray(x, np.float32)).reshape(B * S, D_IN)
    WT3 = np.ascontiguousarray(
        np.asarray(W, np.float32).reshape(OT, 128, D_IN).transpose(0, 2, 1)
    )
    AT2 = np.ascontiguousarray(np.asarray(lora_A, np.float32).T * SCALING)
    BT17 = np.ascontiguousarray(
        np.concatenate(
            [np.asarray(lora_B, np.float32).T, np.asarray(b, np.float32)[None, :]], 0
        )
    )

    in_maps = []
    for c in range(N_CORES):
        xT = np.ascontiguousarray(xf[c * M : (c + 1) * M].T)
        in_maps.append(
            {
                "xT": xT,
                "WT": WT3,
                "AT": AT2,
                "BT": BT17,
                "ones": np.ones((1, M), np.float32),
            }
        )
    return in_maps


def kernel(x, W, b, lora_A, lora_B):
    nc = _build()
    from concourse.bass_utils import run_bass_kernel_spmd

    in_maps = _prep_in_maps(x, W, b, lora_A, lora_B)
    res = run_bass_kernel_spmd(
        nc, in_maps, core_ids=list(range(N_CORES)), trace=False
    )

    out = np.empty((B * S, D_OUT), np.float32)
    for c in range(N_CORES):
        out[c * M : (c + 1) * M] = res.results[c]["outT"].T
    return out.reshape(B, S, D_OUT)
